# revision 4
# baseline (speedup 1.0000x reference)
"""Cross-attention layer on 8 TRN2 NeuronCores.

Sharding: core i -> (batch b = i//2, head-group g = i%2); each core computes
its head-group's contribution to out[b] through Wo; the host sums the two
partial products per batch (row-split of Wo => partial-sum reduction).

Device kernel works in transposed layout ([channels, tokens]) so the softmax
reduction is along the matmul free axis:
  Q^T = Wq_g^T x^T, K^T = Wk_g^T ctx^T, V = ctx Wv_g (+ ones column)
  scores^T_h = K_h Q_h^T  (contraction over head_dim=64)
  E = exp(scores^T/32) * mask^T      (no max subtraction; |scores/32| ~ 1.5)
  U = V'^T E  (per s-tile accumulation; row 64 = softmax denominator)
  O^T = U[0:64] * (1/U[64] broadcast)   (DVE reciprocal + gpsimd broadcast)
  out_partial = O^T^T Wo_g           (host adds core pairs)

PE-array usage (the perf levers over the first working version):
  - scores: K=64 per head; heads 2m / 2m+1 sit in SBUF row halves 0-63 /
    64-127 of qt/kt, so issuing their matmuls back-to-back row-tiles the
    PE (two concurrent 64x128 tiles -> ~2x on the scores matmuls).
  - matmuls are grouped by PE tiling mode (scores = 64x128 row-tiled,
    everything else = full 128x128) at s-tile-pair granularity: mode
    switches drain the PE array, so they are bounded to ~8 per unit.
  - PV(u) and scores(u+1) interleave at s-tile-pair granularity so the PE
    never idles waiting on the activation engine (which paces the exps).
  - 1/den on the DVE (vector) engine: the scalar engine runs only Exp, so
    the per-unit Exp<->Ln activation-table swaps (~40us) are gone.
  - out projection in bf16 instead of fp32r (2x matmul rate, cheap
    LDWEIGHTS); rel-err stays ~5e-3, well inside the 2e-2 gate.

Dtype: attention core in bf16; U accumulates in fp32 PSUM; normalized O^T
and Wo in bf16 with fp32 PSUM accumulation for the final projection.
"""

import os
import numpy as np
import ml_dtypes

import concourse.mybir as mybir
from concourse import bacc
import concourse.tile as tile
from concourse.bass_utils import run_bass_kernel_spmd

B, T, TC = 4, 1024, 1024
C, CTX_C, H = 1024, 1024, 16
HD = C // H            # 64
P = 128
NCORES = 8
HG = 2                 # head groups (core-level split)
HPG = H // HG          # 8 heads per core
HPP = HPG // 2         # 4 head pairs per core
CG = HPG * HD          # 512 channels per group
NT = 512               # matmul free-dim chunk
KO = C // P            # 8 contraction tiles for projections
MQ = CG // P           # 4 partition-tiles of Q^T/K^T (== head pairs)
SO = TC // P           # 8 s-tiles
T2 = T // NT           # 2 t-chunks
KP = CG // P           # 4 contraction tiles for the out projection
NU = HPP * T2          # 8 attention pair-units
F32 = mybir.dt.float32
BF16 = mybir.dt.bfloat16
ALU = mybir.AluOpType
ACTF = mybir.ActivationFunctionType

_CACHED_NC = None


def _ensure_ntff_hook():
    """Register the axon NTFF profiling hook if the image's antenv lacks it."""
    try:
        from antenv.axon_hooks import get_axon_ntff_profile_hook  # noqa: F401
        return
    except ImportError:
        pass
    import sys
    import types
    try:
        from trn_agent_boot.trn_boot import _ntff_profile_via_ctypes
        hook = _ntff_profile_via_ctypes("/opt/axon/libaxon_pjrt.so")
    except Exception:
        hook = None
    mod = types.ModuleType("antenv.axon_hooks")
    mod.get_axon_ntff_profile_hook = lambda: hook
    mod.set_axon_ntff_profile_hook = lambda h: None
    sys.modules["antenv.axon_hooks"] = mod
    import antenv
    antenv.axon_hooks = mod


def _build_program():
    nc = bacc.Bacc("TRN2", target_bir_lowering=False, debug=False,
                   num_devices=NCORES)
    xT = nc.dram_tensor("xT", [C, T], BF16, kind="ExternalInput").ap()
    ctxT = nc.dram_tensor("ctxT", [CTX_C, TC], BF16, kind="ExternalInput").ap()
    maskT = nc.dram_tensor("maskT", [TC, T], BF16, kind="ExternalInput").ap()
    wq = nc.dram_tensor("wq", [C, CG], BF16, kind="ExternalInput").ap()
    wk = nc.dram_tensor("wk", [CTX_C, CG], BF16, kind="ExternalInput").ap()
    wv = nc.dram_tensor("wv", [CTX_C, CG], BF16, kind="ExternalInput").ap()
    wo = nc.dram_tensor("wo", [CG, C], BF16, kind="ExternalInput").ap()
    out = nc.dram_tensor("out", [T, C], F32, kind="ExternalOutput").ap()

    with tile.TileContext(nc) as tc:
        with (
            tc.tile_pool(name="persist", bufs=1) as persist,
            tc.tile_pool(name="work", bufs=3) as work,
            tc.tile_pool(name="etp", bufs=2) as etp,
            tc.tile_pool(name="psmm", bufs=2, space="PSUM") as psmm,
            tc.tile_pool(name="pssc", bufs=4, space="PSUM") as pssc,
            tc.tile_pool(name="psu", bufs=2, space="PSUM") as psu_pool,
        ):
            qt_sb = persist.tile([P, MQ, T], BF16)    # Q^T [(pair,d), t]
            kt_sb = persist.tile([P, MQ, TC], BF16)   # K^T [(pair,d), s]
            vp_sb = persist.tile([P, SO, HPG, HD + 1], BF16)  # V' + ones col
            mask_sb = persist.tile([P, SO, T], BF16)  # mask^T
            ot_sb = persist.tile([P, KP, T], BF16)    # O^T normalized
            wo_sb = persist.tile([P, KP, C], BF16)
            xT_sb = persist.tile([P, KO, T], BF16)
            ctxT_sb = persist.tile([P, KO, TC], BF16)
            wq_sb = persist.tile([P, KO, CG], BF16)
            wk_sb = persist.tile([P, KO, CG], BF16)
            wv_sb = persist.tile([P, KO, CG], BF16)

            nc.gpsimd.memset(vp_sb[:, :, :, HD:HD + 1], 1.0)

            xT_r = xT.rearrange("(ko p) t -> p ko t", p=P)
            ctxT_r = ctxT.rearrange("(ko p) t -> p ko t", p=P)
            nc.sync.dma_start(wq_sb, wq.rearrange("(ko p) m -> p ko m", p=P))
            for kc in range(KO):   # chunked so stage A starts early
                nc.sync.dma_start(xT_sb[:, kc], xT_r[:, kc])
            nc.sync.dma_start(wk_sb, wk.rearrange("(ko p) m -> p ko m", p=P))
            nc.sync.dma_start(wv_sb, wv.rearrange("(ko p) m -> p ko m", p=P))
            for kc in range(KO):
                nc.sync.dma_start(ctxT_sb[:, kc], ctxT_r[:, kc])
            nc.sync.dma_start(mask_sb,
                              maskT.rearrange("(so p) t -> p so t", p=P))
            nc.sync.dma_start(wo_sb, wo.rearrange("(ko p) n -> p ko n", p=P))

            # ---- attention pair-units: u -> (head pair hp, t-chunk t2) ----
            # t2-major so units 0-3 cover t rows 0-511 (first out half)
            def udec(u):
                return u % HPP, u // HPP

            et_tiles = {}
            pv_state = {}

            def scores_pair_so(u, so):
                """Packed scores for heads (2hp, 2hp+1) at s-tile so.

                The two matmuls are adjacent and target row halves 0-63 /
                64-127, so the PE runs them as two concurrent 64x128 tiles.
                """
                hp, t2 = udec(u)
                if so == 0:
                    et_tiles[u] = etp.tile([P, 2, SO, NT], BF16, tag="et",
                                           name=f"et{u}")
                et = et_tiles[u]
                ts = slice(t2 * NT, (t2 + 1) * NT)
                ss = slice(so * P, (so + 1) * P)
                psA = pssc.tile([P, NT], F32, tag="ps_sc")
                psB = pssc.tile([P, NT], F32, tag="ps_sc")
                nc.tensor.matmul(psA, kt_sb[0:64, hp, ss], qt_sb[0:64, hp, ts],
                                 start=True, stop=True)
                nc.tensor.matmul(psB, kt_sb[64:128, hp, ss],
                                 qt_sb[64:128, hp, ts], start=True, stop=True)
                nc.scalar.activation(et[:, 0, so, :], psA, ACTF.Exp,
                                     scale=1.0 / 32.0)
                nc.scalar.activation(et[:, 1, so, :], psB, ACTF.Exp,
                                     scale=1.0 / 32.0)
                if so % 2 == 1:
                    j = so - 1
                    for i in range(2):
                        nc.vector.tensor_tensor(
                            et[:, i, j:j + 2, :], et[:, i, j:j + 2, :],
                            mask_sb[:, j:j + 2, ts], ALU.mult)

            def pv_pair_j(u, j):
                """U chains for both heads of unit u, s-tiles 2j / 2j+1.

                Full 128x128 mode; the two heads accumulate into separate
                PSUM banks (an accumulation group owns its whole bank).
                """
                hp, _ = udec(u)
                if j == 0:
                    pv_state[u] = (
                        psu_pool.tile([HD + 1, NT], F32, tag="ps_u",
                                      name=f"psuA{u}"),
                        psu_pool.tile([HD + 1, NT], F32, tag="ps_u",
                                      name=f"psuB{u}"))
                et = et_tiles[u]
                for i in range(2):
                    psu = pv_state[u][i]
                    for so in (2 * j, 2 * j + 1):
                        nc.tensor.matmul(
                            psu, vp_sb[:, so, 2 * hp + i, :], et[:, i, so, :],
                            start=(so == 0), stop=(so == SO - 1))

            def normalize(u):
                hp, t2 = udec(u)
                psus = pv_state.pop(u)
                del et_tiles[u]
                ts = slice(t2 * NT, (t2 + 1) * NT)
                for i in range(2):
                    psu = psus[i]
                    hs = slice(i * HD, (i + 1) * HD)
                    rc = work.tile([1, NT], F32, tag=f"rc{i}", name=f"rc{i}")
                    nc.vector.reciprocal(rc, psu[HD:HD + 1, :])
                    ucp = work.tile([HD, NT], F32, tag=f"ucp{i}",
                                    name=f"ucp{i}")
                    nc.vector.tensor_copy(ucp, psu[0:HD, :])  # frees the bank
                    bc = work.tile([HD, NT], F32, tag=f"bc{i}", name=f"bc{i}")
                    nc.gpsimd.partition_broadcast(bc, rc)
                    nc.vector.tensor_tensor(ot_sb[hs, hp, ts], ucp, bc,
                                            ALU.mult)

            def out_stage(tms):
                for tm in tms:
                    for c2 in range(C // NT):
                        ps = psmm.tile([P, NT], F32, tag="mm512")
                        for kp in range(KP):
                            nc.tensor.matmul(
                                ps, ot_sb[:, kp, tm * P:(tm + 1) * P],
                                wo_sb[:, kp, c2 * NT:(c2 + 1) * NT],
                                start=(kp == 0), stop=(kp == KP - 1))
                        o_sb = work.tile([P, NT], F32, tag="out")
                        nc.scalar.activation(o_sb, ps, ACTF.Copy)
                        nc.sync.dma_start(
                            out[tm * P:(tm + 1) * P, c2 * NT:(c2 + 1) * NT],
                            o_sb)

            # ---- Stage A/B: projections Q^T, K^T ----
            for m in range(MQ):          # Q^T = Wq^T x^T
                for t2 in range(T2):
                    ps = psmm.tile([P, NT], F32, tag="mm512")
                    for kc in range(KO):
                        nc.tensor.matmul(
                            ps, wq_sb[:, kc, m * P:(m + 1) * P],
                            xT_sb[:, kc, t2 * NT:(t2 + 1) * NT],
                            start=(kc == 0), stop=(kc == KO - 1))
                    nc.vector.tensor_copy(
                        qt_sb[:, m, t2 * NT:(t2 + 1) * NT], ps)
            for m in range(MQ):          # K^T = Wk^T ctx^T
                for s2 in range(T2):
                    ps = psmm.tile([P, NT], F32, tag="mm512")
                    for kc in range(KO):
                        nc.tensor.matmul(
                            ps, wk_sb[:, kc, m * P:(m + 1) * P],
                            ctxT_sb[:, kc, s2 * NT:(s2 + 1) * NT],
                            start=(kc == 0), stop=(kc == KO - 1))
                    nc.vector.tensor_copy(
                        kt_sb[:, m, s2 * NT:(s2 + 1) * NT], ps)
            # V = ctx Wv (natural layout), with unit 0's scores folded in
            for so in range(SO):
                ps = psmm.tile([P, NT], F32, tag="mm512")
                for kc in range(KO):
                    nc.tensor.matmul(
                        ps, ctxT_sb[:, kc, so * P:(so + 1) * P],
                        wv_sb[:, kc, :],
                        start=(kc == 0), stop=(kc == KO - 1))
                nc.vector.tensor_copy(
                    vp_sb[:, so, :, 0:HD],
                    ps.rearrange("p (h d) -> p h d", h=HPG))
                if so % 2 == 1:
                    scores_pair_so(0, so - 1)
                    scores_pair_so(0, so)

            # ---- Stage C: attention; PV(u) / scores(u+1) interleaved at
            # s-tile-pair granularity, matmuls grouped by PE tiling mode ----
            for u in range(NU):
                for j in range(SO // 2):
                    pv_pair_j(u, j)
                    if u + 1 < NU:
                        scores_pair_so(u + 1, 2 * j)
                        scores_pair_so(u + 1, 2 * j + 1)
                normalize(u)
                if u == 4:
                    out_stage(range(0, T // (2 * P)))

            # ---- Stage D: out_partial = O Wo (second half) ----
            out_stage(range(T // (2 * P), T // P))
    nc.compile()
    return nc


def _get_program():
    global _CACHED_NC
    if _CACHED_NC is None:
        _CACHED_NC = _build_program()
    return _CACHED_NC


def kernel(x, context, attn_mask, Wq, Wk, Wv, Wo):
    x = np.asarray(x, dtype=np.float32)
    context = np.asarray(context, dtype=np.float32)
    attn_mask = np.asarray(attn_mask)
    Wq = np.asarray(Wq, dtype=np.float32)
    Wk = np.asarray(Wk, dtype=np.float32)
    Wv = np.asarray(Wv, dtype=np.float32)
    Wo = np.asarray(Wo, dtype=np.float32)

    nc = _get_program()
    bf = ml_dtypes.bfloat16
    in_maps = []
    for i in range(NCORES):
        b, g = i // 2, i % 2
        cs = slice(g * CG, (g + 1) * CG)
        in_maps.append({
            "xT": np.ascontiguousarray(x[b].T).astype(bf),
            "ctxT": np.ascontiguousarray(context[b].T).astype(bf),
            "maskT": np.ascontiguousarray(attn_mask[b, 0].T).astype(bf),
            "wq": np.ascontiguousarray(Wq[:, cs]).astype(bf),
            "wk": np.ascontiguousarray(Wk[:, cs]).astype(bf),
            "wv": np.ascontiguousarray(Wv[:, cs]).astype(bf),
            "wo": np.ascontiguousarray(Wo[cs, :]).astype(bf),
        })

    profile = os.environ.get("KERNEL_PROFILE", "0") == "1"
    if profile:
        _ensure_ntff_hook()
    res = run_bass_kernel_spmd(
        nc, in_maps, list(range(NCORES)),
        trace=profile, trace_cores=[0] if profile else None)
    if profile:
        kernel.last_exec_time_ns = res.exec_time_ns
        kernel.last_trace = res.instructions_and_trace

    out = np.empty((B, T, C), dtype=np.float32)
    for b in range(B):
        out[b] = res.results[2 * b]["out"] + res.results[2 * b + 1]["out"]
    return out


# revision 6
# speedup vs baseline: 1.2237x; 1.2237x over previous
"""Cross-attention layer on 8 TRN2 NeuronCores.

Sharding: core i -> (batch b = i//2, head-group g = i%2); each core computes
its head-group's contribution to out[b] through Wo; the host sums the two
partial products per batch (row-split of Wo => partial-sum reduction).

Device kernel works in transposed layout ([channels, tokens]) so the softmax
reduction is along the matmul free axis:
  Q^T = Wq_g^T x^T, K^T = Wk_g^T ctx^T, V = ctx Wv_g (+ ones column)
  scores^T_h = K_h Q_h^T  (contraction over head_dim=64)
  E = exp(scores^T/32) * mask^T      (no max subtraction; |scores/32| ~ 1.5)
  U = V'^T E  (per s-tile accumulation; row 64 = softmax denominator)
  O^T = U[0:64] * (1/U[64] broadcast)   (DVE reciprocal + gpsimd broadcast)
  out_partial = O^T^T Wo_g           (host adds core pairs)

PE-array usage (the perf levers over the first working version):
  - scores: K=64 per head; heads 2m / 2m+1 sit in SBUF row halves 0-63 /
    64-127 of qt/kt, so issuing their matmuls back-to-back row-tiles the
    PE (two concurrent 64x128 tiles -> ~2x on the scores matmuls).
  - matmuls are grouped by PE tiling mode (scores = 64x128 row-tiled,
    everything else = full 128x128) at s-tile-pair granularity: mode
    switches drain the PE array, so they are bounded to ~8 per unit.
  - PV(u) and scores(u+1) interleave at s-tile-pair granularity so the PE
    never idles waiting on the activation engine (which paces the exps).
  - 1/den on the DVE (vector) engine: the scalar engine runs only Exp, so
    the per-unit Exp<->Ln activation-table swaps (~40us) are gone.
  - out projection in bf16 instead of fp32r (2x matmul rate, cheap
    LDWEIGHTS); rel-err stays ~5e-3, well inside the 2e-2 gate.

Dtype: attention core in bf16; U accumulates in fp32 PSUM; normalized O^T
and Wo in bf16 with fp32 PSUM accumulation for the final projection.
"""

import os
import numpy as np
import ml_dtypes

import concourse.mybir as mybir
from concourse import bacc
import concourse.tile as tile
from concourse.bass_utils import run_bass_kernel_spmd

B, T, TC = 4, 1024, 1024
C, CTX_C, H = 1024, 1024, 16
HD = C // H            # 64
P = 128
NCORES = 8
HG = 2                 # head groups (core-level split)
HPG = H // HG          # 8 heads per core
HPP = HPG // 2         # 4 head pairs per core
CG = HPG * HD          # 512 channels per group
NT = 512               # matmul free-dim chunk
KO = C // P            # 8 contraction tiles for projections
MQ = CG // P           # 4 partition-tiles of Q^T/K^T (== head pairs)
SO = TC // P           # 8 s-tiles
T2 = T // NT           # 2 t-chunks
KP = CG // P           # 4 contraction tiles for the out projection
NU = HPP * T2          # 8 attention pair-units
F32 = mybir.dt.float32
BF16 = mybir.dt.bfloat16
ALU = mybir.AluOpType
ACTF = mybir.ActivationFunctionType

_CACHED_NC = None


def _ensure_ntff_hook():
    """Register the axon NTFF profiling hook if the image's antenv lacks it."""
    try:
        from antenv.axon_hooks import get_axon_ntff_profile_hook  # noqa: F401
        return
    except ImportError:
        pass
    import sys
    import types
    try:
        from trn_agent_boot.trn_boot import _ntff_profile_via_ctypes
        hook = _ntff_profile_via_ctypes("/opt/axon/libaxon_pjrt.so")
    except Exception:
        hook = None
    mod = types.ModuleType("antenv.axon_hooks")
    mod.get_axon_ntff_profile_hook = lambda: hook
    mod.set_axon_ntff_profile_hook = lambda h: None
    sys.modules["antenv.axon_hooks"] = mod
    import antenv
    antenv.axon_hooks = mod


def _build_program():
    nc = bacc.Bacc("TRN2", target_bir_lowering=False, debug=False,
                   num_devices=NCORES)
    xT = nc.dram_tensor("xT", [C, T], BF16, kind="ExternalInput").ap()
    ctxT = nc.dram_tensor("ctxT", [CTX_C, TC], BF16, kind="ExternalInput").ap()
    maskT = nc.dram_tensor("maskT", [TC, T], BF16, kind="ExternalInput").ap()
    wq = nc.dram_tensor("wq", [C, CG], BF16, kind="ExternalInput").ap()
    wk = nc.dram_tensor("wk", [CTX_C, CG], BF16, kind="ExternalInput").ap()
    wv = nc.dram_tensor("wv", [CTX_C, CG], BF16, kind="ExternalInput").ap()
    wo = nc.dram_tensor("wo", [CG, C], BF16, kind="ExternalInput").ap()
    out = nc.dram_tensor("out", [T, C], F32, kind="ExternalOutput").ap()

    with tile.TileContext(nc) as tc:
        with (
            tc.tile_pool(name="persist", bufs=1) as persist,
            tc.tile_pool(name="work", bufs=3) as work,
            tc.tile_pool(name="etp", bufs=2) as etp,
            tc.tile_pool(name="psmm", bufs=2, space="PSUM") as psmm,
            tc.tile_pool(name="pssc", bufs=2, space="PSUM") as pssc,
            tc.tile_pool(name="psu", bufs=2, space="PSUM") as psu_pool,
        ):
            qt_sb = persist.tile([P, MQ, T], BF16)    # Q^T [(pair,d), t]
            kt_sb = persist.tile([P, MQ, TC], BF16)   # K^T [(pair,d), s]
            vp_sb = persist.tile([P, SO, HPG, HD + 1], BF16)  # V' + ones col
            mask_sb = persist.tile([P, SO, T], BF16)  # mask^T
            ot_sb = persist.tile([P, KP, T], BF16)    # O^T normalized
            wo_sb = persist.tile([P, KP, C], BF16)
            xT_sb = persist.tile([P, KO, T], BF16)
            ctxT_sb = persist.tile([P, KO, TC], BF16)
            wq_sb = persist.tile([P, KO, CG], BF16)
            wk_sb = persist.tile([P, KO, CG], BF16)
            wv_sb = persist.tile([P, KO, CG], BF16)

            nc.gpsimd.memset(vp_sb[:, :, :, HD:HD + 1], 1.0)

            xT_r = xT.rearrange("(ko p) t -> p ko t", p=P)
            ctxT_r = ctxT.rearrange("(ko p) t -> p ko t", p=P)
            nc.sync.dma_start(wq_sb, wq.rearrange("(ko p) m -> p ko m", p=P))
            for kc in range(KO):   # chunked so stage A starts early
                nc.sync.dma_start(xT_sb[:, kc], xT_r[:, kc])
            nc.sync.dma_start(wk_sb, wk.rearrange("(ko p) m -> p ko m", p=P))
            nc.sync.dma_start(wv_sb, wv.rearrange("(ko p) m -> p ko m", p=P))
            for kc in range(KO):
                nc.sync.dma_start(ctxT_sb[:, kc], ctxT_r[:, kc])
            nc.sync.dma_start(mask_sb,
                              maskT.rearrange("(so p) t -> p so t", p=P))
            nc.sync.dma_start(wo_sb, wo.rearrange("(ko p) n -> p ko n", p=P))

            # ---- attention pair-units: u -> (head pair hp, t-chunk t2) ----
            # t2-major so units 0-3 cover t rows 0-511 (first out half)
            def udec(u):
                return u % HPP, u // HPP

            et_tiles = {}
            pv_state = {}

            def scores_pair_j(u, j):
                """Packed scores for heads (2hp, 2hp+1), s-tiles 2j / 2j+1.

                Per s-tile the two head matmuls are adjacent and target row
                halves 0-63 / 64-127, so the PE runs them as two concurrent
                64x128 tiles; both land in one 2-bank PSUM tile drained by a
                single [128,1024] Exp (halves the activation-engine load).
                """
                hp, t2 = udec(u)
                if j == 0:
                    et_tiles[u] = etp.tile([P, SO, 2, NT], BF16, tag="et",
                                           name=f"et{u}")
                et = et_tiles[u]
                ts = slice(t2 * NT, (t2 + 1) * NT)
                for so in (2 * j, 2 * j + 1):
                    ss = slice(so * P, (so + 1) * P)
                    ps2 = pssc.tile([P, 2, NT], F32, tag="ps_sc",
                                    name="ps_sc")
                    nc.tensor.matmul(ps2[:, 0, :], kt_sb[0:64, hp, ss],
                                     qt_sb[0:64, hp, ts],
                                     start=True, stop=True)
                    nc.tensor.matmul(ps2[:, 1, :], kt_sb[64:128, hp, ss],
                                     qt_sb[64:128, hp, ts],
                                     start=True, stop=True)
                    nc.scalar.activation(et[:, so], ps2, ACTF.Exp,
                                         scale=1.0 / 32.0)
                for i in range(2):
                    nc.vector.tensor_tensor(
                        et[:, 2 * j:2 * j + 2, i, :],
                        et[:, 2 * j:2 * j + 2, i, :],
                        mask_sb[:, 2 * j:2 * j + 2, ts], ALU.mult)

            def pv_pair_j(u, j):
                """U chains for both heads of unit u, s-tiles 2j / 2j+1.

                Full 128x128 mode; the two heads accumulate into separate
                PSUM banks (an accumulation group owns its whole bank).
                """
                hp, _ = udec(u)
                if j == 0:
                    pv_state[u] = (
                        psu_pool.tile([HD + 1, NT], F32, tag="ps_u",
                                      name=f"psuA{u}"),
                        psu_pool.tile([HD + 1, NT], F32, tag="ps_u",
                                      name=f"psuB{u}"))
                et = et_tiles[u]
                for i in range(2):
                    psu = pv_state[u][i]
                    for so in (2 * j, 2 * j + 1):
                        nc.tensor.matmul(
                            psu, vp_sb[:, so, 2 * hp + i, :], et[:, so, i, :],
                            start=(so == 0), stop=(so == SO - 1))

            def normalize(u):
                hp, t2 = udec(u)
                psus = pv_state.pop(u)
                del et_tiles[u]
                ts = slice(t2 * NT, (t2 + 1) * NT)
                for i in range(2):
                    psu = psus[i]
                    hs = slice(i * HD, (i + 1) * HD)
                    # one copy frees the PSUM bank; recip runs on the copy
                    ucp = work.tile([HD + 1, NT], F32, tag=f"ucp{i}",
                                    name=f"ucp{i}")
                    nc.scalar.activation(ucp, psu, ACTF.Copy)
                    rc = work.tile([1, NT], F32, tag=f"rc{i}", name=f"rc{i}")
                    nc.vector.reciprocal(rc, ucp[HD:HD + 1, :])
                    bc = work.tile([HD, NT], F32, tag=f"bc{i}", name=f"bc{i}")
                    nc.gpsimd.partition_broadcast(bc, rc)
                    nc.vector.tensor_tensor(ot_sb[hs, hp, ts], ucp[0:HD, :],
                                            bc, ALU.mult)

            def out_stage(tms):
                for tm in tms:
                    for c2 in range(C // NT):
                        ps = psmm.tile([P, NT], F32, tag="mm512")
                        for kp in range(KP):
                            nc.tensor.matmul(
                                ps, ot_sb[:, kp, tm * P:(tm + 1) * P],
                                wo_sb[:, kp, c2 * NT:(c2 + 1) * NT],
                                start=(kp == 0), stop=(kp == KP - 1))
                        o_sb = work.tile([P, NT], F32, tag="out")
                        nc.scalar.activation(o_sb, ps, ACTF.Copy)
                        nc.sync.dma_start(
                            out[tm * P:(tm + 1) * P, c2 * NT:(c2 + 1) * NT],
                            o_sb)

            # ---- Stage A/B: projections Q^T, K^T ----
            for m in range(MQ):          # Q^T = Wq^T x^T
                for t2 in range(T2):
                    ps = psmm.tile([P, NT], F32, tag="mm512")
                    for kc in range(KO):
                        nc.tensor.matmul(
                            ps, wq_sb[:, kc, m * P:(m + 1) * P],
                            xT_sb[:, kc, t2 * NT:(t2 + 1) * NT],
                            start=(kc == 0), stop=(kc == KO - 1))
                    nc.scalar.activation(
                        qt_sb[:, m, t2 * NT:(t2 + 1) * NT], ps, ACTF.Copy)
            for m in range(MQ):          # K^T = Wk^T ctx^T
                for s2 in range(T2):
                    ps = psmm.tile([P, NT], F32, tag="mm512")
                    for kc in range(KO):
                        nc.tensor.matmul(
                            ps, wk_sb[:, kc, m * P:(m + 1) * P],
                            ctxT_sb[:, kc, s2 * NT:(s2 + 1) * NT],
                            start=(kc == 0), stop=(kc == KO - 1))
                    nc.scalar.activation(
                        kt_sb[:, m, s2 * NT:(s2 + 1) * NT], ps, ACTF.Copy)
            # V = ctx Wv (natural layout), with unit 0's scores folded in
            for so in range(SO):
                ps = psmm.tile([P, NT], F32, tag="mm512")
                for kc in range(KO):
                    nc.tensor.matmul(
                        ps, ctxT_sb[:, kc, so * P:(so + 1) * P],
                        wv_sb[:, kc, :],
                        start=(kc == 0), stop=(kc == KO - 1))
                nc.scalar.activation(
                    vp_sb[:, so, :, 0:HD],
                    ps.rearrange("p (h d) -> p h d", h=HPG), ACTF.Copy)
                if so % 2 == 1:
                    scores_pair_j(0, so // 2)

            # ---- Stage C: attention; PV(u) / scores(u+1) interleaved at
            # s-tile-pair granularity, matmuls grouped by PE tiling mode ----
            for u in range(NU):
                for j in range(SO // 2):
                    pv_pair_j(u, j)
                    if u + 1 < NU:
                        scores_pair_j(u + 1, j)
                normalize(u)
                if u == 4:
                    out_stage(range(0, T // (2 * P)))

            # ---- Stage D: out_partial = O Wo (second half) ----
            out_stage(range(T // (2 * P), T // P))
    nc.compile()
    return nc


def _get_program():
    global _CACHED_NC
    if _CACHED_NC is None:
        _CACHED_NC = _build_program()
    return _CACHED_NC


def kernel(x, context, attn_mask, Wq, Wk, Wv, Wo):
    x = np.asarray(x, dtype=np.float32)
    context = np.asarray(context, dtype=np.float32)
    attn_mask = np.asarray(attn_mask)
    Wq = np.asarray(Wq, dtype=np.float32)
    Wk = np.asarray(Wk, dtype=np.float32)
    Wv = np.asarray(Wv, dtype=np.float32)
    Wo = np.asarray(Wo, dtype=np.float32)

    nc = _get_program()
    bf = ml_dtypes.bfloat16
    in_maps = []
    for i in range(NCORES):
        b, g = i // 2, i % 2
        cs = slice(g * CG, (g + 1) * CG)
        in_maps.append({
            "xT": np.ascontiguousarray(x[b].T).astype(bf),
            "ctxT": np.ascontiguousarray(context[b].T).astype(bf),
            "maskT": np.ascontiguousarray(attn_mask[b, 0].T).astype(bf),
            "wq": np.ascontiguousarray(Wq[:, cs]).astype(bf),
            "wk": np.ascontiguousarray(Wk[:, cs]).astype(bf),
            "wv": np.ascontiguousarray(Wv[:, cs]).astype(bf),
            "wo": np.ascontiguousarray(Wo[cs, :]).astype(bf),
        })

    profile = os.environ.get("KERNEL_PROFILE", "0") == "1"
    if profile:
        _ensure_ntff_hook()
    res = run_bass_kernel_spmd(
        nc, in_maps, list(range(NCORES)),
        trace=profile, trace_cores=[0] if profile else None)
    if profile:
        kernel.last_exec_time_ns = res.exec_time_ns
        kernel.last_trace = res.instructions_and_trace

    out = np.empty((B, T, C), dtype=np.float32)
    for b in range(B):
        out[b] = res.results[2 * b]["out"] + res.results[2 * b + 1]["out"]
    return out


# revision 7
# speedup vs baseline: 1.3699x; 1.1195x over previous
"""Cross-attention layer on 8 TRN2 NeuronCores.

Sharding: core i -> (batch b = i//2, head-group g = i%2); each core computes
its head-group's contribution to out[b] through Wo; the host sums the two
partial products per batch (row-split of Wo => partial-sum reduction).

Device kernel works in transposed layout ([channels, tokens]) so the softmax
reduction is along the matmul free axis:
  Q^T = Wq_g^T x^T, K^T = Wk_g^T ctx^T, V = ctx Wv_g (+ ones column)
  scores^T_h = K_h Q_h^T  (contraction over head_dim=64)
  E = exp(scores^T/32) * mask^T      (no max subtraction; |scores/32| ~ 1.5)
  U = V'^T E  (per s-tile accumulation; row 64 = softmax denominator)
  O^T = U[0:64] * (1/U[64] broadcast)   (DVE reciprocal + gpsimd broadcast)
  out_partial = O^T^T Wo_g           (host adds core pairs)

PE-array usage (the perf levers over the first working version):
  - scores: K=64 per head; heads 2m / 2m+1 sit in SBUF row halves 0-63 /
    64-127 of qt/kt, so issuing their matmuls back-to-back row-tiles the
    PE (two concurrent 64x128 tiles -> ~2x on the scores matmuls).
  - matmuls are grouped by PE tiling mode (scores = 64x128 row-tiled,
    everything else = full 128x128) at s-tile-pair granularity: mode
    switches drain the PE array, so they are bounded to ~8 per unit.
  - PV(u) and scores(u+1) interleave at s-tile-pair granularity so the PE
    never idles waiting on the activation engine (which paces the exps).
  - 1/den on the DVE (vector) engine: the scalar engine runs only Exp, so
    the per-unit Exp<->Ln activation-table swaps (~40us) are gone.
  - out projection in bf16 instead of fp32r (2x matmul rate, cheap
    LDWEIGHTS); rel-err stays ~5e-3, well inside the 2e-2 gate.

Dtype: attention core in bf16; U accumulates in fp32 PSUM; normalized O^T
and Wo in bf16 with fp32 PSUM accumulation for the final projection.
"""

import os
import numpy as np
import ml_dtypes

import concourse.mybir as mybir
from concourse import bacc
import concourse.tile as tile
from concourse.bass_utils import run_bass_kernel_spmd

B, T, TC = 4, 1024, 1024
C, CTX_C, H = 1024, 1024, 16
HD = C // H            # 64
P = 128
NCORES = 8
HG = 2                 # head groups (core-level split)
HPG = H // HG          # 8 heads per core
HPP = HPG // 2         # 4 head pairs per core
CG = HPG * HD          # 512 channels per group
NT = 512               # matmul free-dim chunk
KO = C // P            # 8 contraction tiles for projections
MQ = CG // P           # 4 partition-tiles of Q^T/K^T (== head pairs)
SO = TC // P           # 8 s-tiles
T2 = T // NT           # 2 t-chunks
KP = CG // P           # 4 contraction tiles for the out projection
NU = HPP * T2          # 8 attention pair-units
F32 = mybir.dt.float32
BF16 = mybir.dt.bfloat16
ALU = mybir.AluOpType
ACTF = mybir.ActivationFunctionType

_CACHED_NC = None


def _ensure_ntff_hook():
    """Register the axon NTFF profiling hook if the image's antenv lacks it."""
    try:
        from antenv.axon_hooks import get_axon_ntff_profile_hook  # noqa: F401
        return
    except ImportError:
        pass
    import sys
    import types
    try:
        from trn_agent_boot.trn_boot import _ntff_profile_via_ctypes
        hook = _ntff_profile_via_ctypes("/opt/axon/libaxon_pjrt.so")
    except Exception:
        hook = None
    mod = types.ModuleType("antenv.axon_hooks")
    mod.get_axon_ntff_profile_hook = lambda: hook
    mod.set_axon_ntff_profile_hook = lambda h: None
    sys.modules["antenv.axon_hooks"] = mod
    import antenv
    antenv.axon_hooks = mod


def _build_program():
    nc = bacc.Bacc("TRN2", target_bir_lowering=False, debug=False,
                   num_devices=NCORES)
    xT = nc.dram_tensor("xT", [C, T], BF16, kind="ExternalInput").ap()
    ctxT = nc.dram_tensor("ctxT", [CTX_C, TC], BF16, kind="ExternalInput").ap()
    maskT = nc.dram_tensor("maskT", [TC, T], BF16, kind="ExternalInput").ap()
    wq = nc.dram_tensor("wq", [C, CG], BF16, kind="ExternalInput").ap()
    wk = nc.dram_tensor("wk", [CTX_C, CG], BF16, kind="ExternalInput").ap()
    wv = nc.dram_tensor("wv", [CTX_C, CG], BF16, kind="ExternalInput").ap()
    wo = nc.dram_tensor("wo", [CG, C], BF16, kind="ExternalInput").ap()
    out = nc.dram_tensor("out", [T, C], F32, kind="ExternalOutput").ap()

    with tile.TileContext(nc) as tc:
        with (
            tc.tile_pool(name="persist", bufs=1) as persist,
            tc.tile_pool(name="work", bufs=3) as work,
            tc.tile_pool(name="etp", bufs=2) as etp,
            tc.tile_pool(name="psmm", bufs=2, space="PSUM") as psmm,
            tc.tile_pool(name="pssc", bufs=2, space="PSUM") as pssc,
            tc.tile_pool(name="psu", bufs=2, space="PSUM") as psu_pool,
        ):
            qt_sb = persist.tile([P, MQ, T], BF16)    # Q^T [(pair,d), t]
            kt_sb = persist.tile([P, MQ, TC], BF16)   # K^T [(pair,d), s]
            vp_sb = persist.tile([P, SO, HPG, HD + 1], BF16)  # V' + ones col
            mask_sb = persist.tile([P, SO, T], BF16)  # mask^T
            ot_sb = persist.tile([P, KP, T], BF16)    # O^T normalized
            wo_sb = persist.tile([P, KP, C], BF16)
            xT_sb = persist.tile([P, KO, T], BF16)
            ctxT_sb = persist.tile([P, KO, TC], BF16)
            wq_sb = persist.tile([P, KO, CG], BF16)
            wk_sb = persist.tile([P, KO, CG], BF16)
            wv_sb = persist.tile([P, KO, CG], BF16)

            nc.gpsimd.memset(vp_sb[:, :, :, HD:HD + 1], 1.0)

            xT_r = xT.rearrange("(ko p) t -> p ko t", p=P)
            ctxT_r = ctxT.rearrange("(ko p) t -> p ko t", p=P)
            nc.sync.dma_start(wq_sb, wq.rearrange("(ko p) m -> p ko m", p=P))
            for kc in range(KO):   # chunked so stage A starts early
                nc.sync.dma_start(xT_sb[:, kc], xT_r[:, kc])
            nc.sync.dma_start(wk_sb, wk.rearrange("(ko p) m -> p ko m", p=P))
            nc.sync.dma_start(wv_sb, wv.rearrange("(ko p) m -> p ko m", p=P))
            for kc in range(KO):
                nc.sync.dma_start(ctxT_sb[:, kc], ctxT_r[:, kc])
            nc.sync.dma_start(mask_sb,
                              maskT.rearrange("(so p) t -> p so t", p=P))
            nc.sync.dma_start(wo_sb, wo.rearrange("(ko p) n -> p ko n", p=P))

            # ---- attention pair-units: u -> (head pair hp, t-chunk t2) ----
            # t2-major so units 0-3 cover t rows 0-511 (first out half)
            def udec(u):
                return u % HPP, u // HPP

            et_tiles = {}
            pv_state = {}

            def scores_pair_j(u, j):
                """Packed scores for heads (2hp, 2hp+1), s-tiles 2j / 2j+1.

                Per s-tile the two head matmuls are adjacent and target row
                halves 0-63 / 64-127, so the PE runs them as two concurrent
                64x128 tiles; both land in one 2-bank PSUM tile drained by a
                single [128,1024] Exp (halves the activation-engine load).
                """
                hp, t2 = udec(u)
                if j == 0:
                    et_tiles[u] = etp.tile([P, SO, 2, NT], BF16, tag="et",
                                           name=f"et{u}")
                et = et_tiles[u]
                ts = slice(t2 * NT, (t2 + 1) * NT)
                for so in (2 * j, 2 * j + 1):
                    ss = slice(so * P, (so + 1) * P)
                    ps2 = pssc.tile([P, 2, NT], F32, tag="ps_sc",
                                    name="ps_sc")
                    nc.tensor.matmul(ps2[:, 0, :], kt_sb[0:64, hp, ss],
                                     qt_sb[0:64, hp, ts],
                                     start=True, stop=True)
                    nc.tensor.matmul(ps2[:, 1, :], kt_sb[64:128, hp, ss],
                                     qt_sb[64:128, hp, ts],
                                     start=True, stop=True)
                    nc.scalar.activation(et[:, so], ps2, ACTF.Exp,
                                         scale=1.0 / 32.0)
                for i in range(2):
                    nc.vector.tensor_tensor(
                        et[:, 2 * j:2 * j + 2, i, :],
                        et[:, 2 * j:2 * j + 2, i, :],
                        mask_sb[:, 2 * j:2 * j + 2, ts], ALU.mult)

            def pv_pair_j(u, j):
                """U chains for both heads of unit u, s-tiles 2j / 2j+1.

                Full 128x128 mode; the two heads accumulate into separate
                PSUM banks (an accumulation group owns its whole bank).
                """
                hp, _ = udec(u)
                if j == 0:
                    pv_state[u] = (
                        psu_pool.tile([HD + 1, NT], F32, tag="ps_u",
                                      name=f"psuA{u}"),
                        psu_pool.tile([HD + 1, NT], F32, tag="ps_u",
                                      name=f"psuB{u}"))
                et = et_tiles[u]
                for i in range(2):
                    psu = pv_state[u][i]
                    for so in (2 * j, 2 * j + 1):
                        nc.tensor.matmul(
                            psu, vp_sb[:, so, 2 * hp + i, :], et[:, so, i, :],
                            start=(so == 0), stop=(so == SO - 1))

            def normalize(u):
                hp, t2 = udec(u)
                psus = pv_state.pop(u)
                del et_tiles[u]
                ts = slice(t2 * NT, (t2 + 1) * NT)
                for i in range(2):
                    psu = psus[i]
                    hs = slice(i * HD, (i + 1) * HD)
                    # copies free the PSUM bank fast; the den row goes to a
                    # partition-0 tile (reciprocal_approx_fast NaNs on HW
                    # when its input has a nonzero base partition)
                    ucp = work.tile([HD, NT], F32, tag=f"ucp{i}",
                                    name=f"ucp{i}")
                    nc.scalar.activation(ucp, psu[0:HD, :], ACTF.Copy)
                    dcp = work.tile([1, NT], F32, tag=f"dcp{i}",
                                    name=f"dcp{i}")
                    nc.vector.tensor_copy(dcp, psu[HD:HD + 1, :])
                    rc = work.tile([1, NT], F32, tag=f"rc{i}", name=f"rc{i}")
                    nc.vector.reciprocal_approx_fast(rc, dcp)
                    bc = work.tile([HD, NT], F32, tag=f"bc{i}", name=f"bc{i}")
                    nc.gpsimd.partition_broadcast(bc, rc)
                    nc.vector.tensor_tensor(ot_sb[hs, hp, ts], ucp, bc,
                                            ALU.mult)

            def out_stage(tms):
                for tm in tms:
                    for c2 in range(C // NT):
                        ps = psmm.tile([P, NT], F32, tag="mm512")
                        for kp in range(KP):
                            nc.tensor.matmul(
                                ps, ot_sb[:, kp, tm * P:(tm + 1) * P],
                                wo_sb[:, kp, c2 * NT:(c2 + 1) * NT],
                                start=(kp == 0), stop=(kp == KP - 1))
                        o_sb = work.tile([P, NT], F32, tag="out")
                        nc.scalar.activation(o_sb, ps, ACTF.Copy)
                        nc.sync.dma_start(
                            out[tm * P:(tm + 1) * P, c2 * NT:(c2 + 1) * NT],
                            o_sb)

            # ---- Stage A/B: projections Q^T, K^T ----
            for m in range(MQ):          # Q^T = Wq^T x^T
                for t2 in range(T2):
                    ps = psmm.tile([P, NT], F32, tag="mm512")
                    for kc in range(KO):
                        nc.tensor.matmul(
                            ps, wq_sb[:, kc, m * P:(m + 1) * P],
                            xT_sb[:, kc, t2 * NT:(t2 + 1) * NT],
                            start=(kc == 0), stop=(kc == KO - 1))
                    nc.scalar.activation(
                        qt_sb[:, m, t2 * NT:(t2 + 1) * NT], ps, ACTF.Copy)
            for m in range(MQ):          # K^T = Wk^T ctx^T
                for s2 in range(T2):
                    ps = psmm.tile([P, NT], F32, tag="mm512")
                    for kc in range(KO):
                        nc.tensor.matmul(
                            ps, wk_sb[:, kc, m * P:(m + 1) * P],
                            ctxT_sb[:, kc, s2 * NT:(s2 + 1) * NT],
                            start=(kc == 0), stop=(kc == KO - 1))
                    nc.scalar.activation(
                        kt_sb[:, m, s2 * NT:(s2 + 1) * NT], ps, ACTF.Copy)
            # V = ctx Wv (natural layout), with unit 0's scores folded in
            for so in range(SO):
                ps = psmm.tile([P, NT], F32, tag="mm512")
                for kc in range(KO):
                    nc.tensor.matmul(
                        ps, ctxT_sb[:, kc, so * P:(so + 1) * P],
                        wv_sb[:, kc, :],
                        start=(kc == 0), stop=(kc == KO - 1))
                nc.scalar.activation(
                    vp_sb[:, so, :, 0:HD],
                    ps.rearrange("p (h d) -> p h d", h=HPG), ACTF.Copy)
                if so % 2 == 1:
                    scores_pair_j(0, so // 2)

            # ---- Stage C: attention; PV(u) / scores(u+1) interleaved at
            # s-tile-pair granularity, matmuls grouped by PE tiling mode ----
            for u in range(NU):
                for j in range(SO // 2):
                    pv_pair_j(u, j)
                    if u + 1 < NU:
                        scores_pair_j(u + 1, j)
                normalize(u)
                if u == 4:
                    out_stage(range(0, T // (2 * P)))

            # ---- Stage D: out_partial = O Wo (second half) ----
            out_stage(range(T // (2 * P), T // P))
    nc.compile()
    return nc


def _get_program():
    global _CACHED_NC
    if _CACHED_NC is None:
        _CACHED_NC = _build_program()
    return _CACHED_NC


def kernel(x, context, attn_mask, Wq, Wk, Wv, Wo):
    x = np.asarray(x, dtype=np.float32)
    context = np.asarray(context, dtype=np.float32)
    attn_mask = np.asarray(attn_mask)
    Wq = np.asarray(Wq, dtype=np.float32)
    Wk = np.asarray(Wk, dtype=np.float32)
    Wv = np.asarray(Wv, dtype=np.float32)
    Wo = np.asarray(Wo, dtype=np.float32)

    nc = _get_program()
    bf = ml_dtypes.bfloat16
    in_maps = []
    for i in range(NCORES):
        b, g = i // 2, i % 2
        cs = slice(g * CG, (g + 1) * CG)
        in_maps.append({
            "xT": np.ascontiguousarray(x[b].T).astype(bf),
            "ctxT": np.ascontiguousarray(context[b].T).astype(bf),
            "maskT": np.ascontiguousarray(attn_mask[b, 0].T).astype(bf),
            "wq": np.ascontiguousarray(Wq[:, cs]).astype(bf),
            "wk": np.ascontiguousarray(Wk[:, cs]).astype(bf),
            "wv": np.ascontiguousarray(Wv[:, cs]).astype(bf),
            "wo": np.ascontiguousarray(Wo[cs, :]).astype(bf),
        })

    profile = os.environ.get("KERNEL_PROFILE", "0") == "1"
    if profile:
        _ensure_ntff_hook()
    res = run_bass_kernel_spmd(
        nc, in_maps, list(range(NCORES)),
        trace=profile, trace_cores=[0] if profile else None)
    if profile:
        kernel.last_exec_time_ns = res.exec_time_ns
        kernel.last_trace = res.instructions_and_trace

    out = np.empty((B, T, C), dtype=np.float32)
    for b in range(B):
        out[b] = res.results[2 * b]["out"] + res.results[2 * b + 1]["out"]
    return out
